# revision 8
# baseline (speedup 1.0000x reference)
"""LookAheadMask kernel for Trainium2 — in-place, pure-write, staircase diag.

out[b, r, c] = 1.0 if c > r else x[b, r, c], for x of shape (8, 4096, 4096) f32.

Sharding: batch dim across 8 NeuronCores (data parallel, no communication).

The output aliases the input buffer (lowering_input_output_aliases={0: 0}
through the BIR-lowering/NKI path), so everything at/below the diagonal
never moves. This version writes ONLY the strictly-upper triangle —
33.55 MB per core, ZERO reads (v1 gathered 4.2 MB of diag windows and
re-wrote ~1 MB of at/below-diag bytes through an affine_select path,
serializing ~30 us of descriptor-head work before the streams started).

Empirical DMA model (v1 trace, all 16 engines, all 8 cores live):
desc cost ~ 6 ns + bytes/27.3 B/ns per engine; pool peak ~437 GB/s/core.

  - Bulk: per 128-row block b (0..30), a [128 x (3968-128b)] ones
    rectangle covers cols >= blockstart+128. 32.5 MB, 3968 descriptors,
    bytes split exactly 50/50 between the SP and ACT HWDGE queues.
  - The strict upper triangles of the 32 diagonal 128x128 blocks are
    written by a 7-level binary staircase decomposition: level l has
    32*2^l square rects of size 64/2^l, uniformly spaced at stride
    (128>>l)*(S+1) across ALL blocks -> ONE dma_start per level. All 7
    go through the gpsimd SWDGE queue (cheap issue + sw desc-gen), so
    their 14336 small descriptors trickle into the engine pool under
    the bulk streams.
  - Two-stage ones memset: [:, :2048] first (covers every staircase
    source window and the narrow bulk blocks), then the rest, so DMA
    starts at ~0.3 us instead of ~4 us.
"""

import numpy as np

S = 4096
P = 128
N_CORES = 8
ONES_W = 3968  # widest bulk rectangle (block 0)
M1_W = 2048  # first-stage memset width

# Bulk blocks b=0..30 write out[128b:128b+128, 128b+128:4096] (width
# 3968-128b). This byte-split is exactly 50/50 (pairs (4k,4k+3)/(4k+1,4k+2));
# block 30 goes through the gpsimd SWDGE queue as a probe of the third head.
SP_BULK = [0, 3, 4, 7, 8, 11, 12, 15, 16, 19, 20, 23, 24, 27, 28]
ACT_BULK = [1, 2, 5, 6, 9, 10, 13, 14, 17, 18, 21, 22, 25, 26, 29]
GP_BULK = [30]

# Diagonal-triangle work split: binary staircase levels 0,1 (w=64,32) plus
# ragged per-row-of-32-group writes (31 row groups, lengths 31..1). ACT's
# HWDGE head is ~3x slower per descriptor than SP's, so it only takes 8 of
# the 31 ragged groups.
ACT_RAGGED = [0, 4, 8, 12, 16, 20, 24, 28]
SP_RAGGED = [m for m in range(31) if m not in ACT_RAGGED]

_cached = None


def _build():
    from concourse import bass, mybir

    nc = bass.Bass(target_bir_lowering=True, enable_partition_id=False)
    x = nc.dram_tensor("x", [S, S], mybir.dt.float32, kind="ExternalInput")
    out = nc.dram_tensor("out", [S, S], mybir.dt.float32, kind="ExternalOutput")

    N_WRITES = (
        len(SP_BULK) + len(ACT_BULK) + len(GP_BULK) + 2 + 31
    )  # 64 dma_starts

    def bulk(eng, blocks, ones, dsem, narrow=None):
        for b in blocks:
            if narrow is not None and narrow != (b >= 15):
                continue
            r0 = b * P
            w = S - r0 - P
            eng.dma_start(
                out=out[r0 : r0 + P, r0 + P : S], in_=ones[:, :w]
            ).then_inc(dsem, 16)

    def level(eng, l, ones, dsem):
        # Binary staircase level l of the strict upper triangles of the 32
        # diagonal 128x128 blocks: n = 32<<l rects of h = 64>>l at stride
        # (128>>l)*(S+1). Partition r of rect k sources ones[r, k*h:k*h+h]
        # (n*h == 2048, inside the first-stage memset).
        s = 128 >> l
        h = s >> 1
        n = 32 << l
        eng.dma_start(
            out=bass.AP(out, h, [[S, h], [s * (S + 1), n], [1, h]]),
            in_=bass.AP(ones, 0, [[ONES_W, h], [h, n], [1, h]]),
        ).then_inc(dsem, 16)

    def ragged(eng, ms, ones, dsem):
        # Row j = m of each 32-row group covers cols [j+1, 32) of the
        # group-diagonal 32x32 block: 128 descriptors of L = 31-m f32.
        for m in ms:
            L = 31 - m
            with nc.allow_non_contiguous_dma(
                reason="last ragged group writes isolated single f32 cells"
            ):
                eng.dma_start(
                    out=bass.AP(
                        out, 1 + m * (S + 1), [[32 * (S + 1), 128], [1, L]]
                    ),
                    in_=ones[:, :L],
                ).then_inc(dsem, 16)

    with (
        nc.Block() as block,
        nc.semaphore("dsem") as dsem,  # all output-write DMA completions
        nc.semaphore("m1") as m1,  # ones[:, :2048] memset done
        nc.semaphore("msem") as msem,  # full ones memset done
        nc.sbuf_tensor("ones", [P, ONES_W], mybir.dt.float32) as ones,
    ):

        @block.vector
        def _(vector: bass.BassVectorEngine):
            vector.memset(ones[:, :M1_W], 1.0).then_inc(m1, 1)
            vector.memset(ones[:, M1_W:], 1.0).then_inc(msem, 1)

        @block.sync
        def _(sync: bass.BassEngine):
            sync.wait_ge(m1, 1)
            bulk(sync, SP_BULK, ones, dsem, narrow=True)
            level(sync, 0, ones, dsem)
            level(sync, 1, ones, dsem)
            ragged(sync, SP_RAGGED[:8], ones, dsem)
            sync.wait_ge(msem, 1)
            bulk(sync, [0, 3], ones, dsem)
            ragged(sync, SP_RAGGED[8:16], ones, dsem)
            bulk(sync, [4, 7], ones, dsem)
            ragged(sync, SP_RAGGED[16:], ones, dsem)
            bulk(sync, [8, 11, 12], ones, dsem)
            sync.wait_ge(dsem, 16 * N_WRITES)

        @block.scalar
        def _(scalar: bass.BassEngine):
            scalar.wait_ge(m1, 1)
            bulk(scalar, ACT_BULK, ones, dsem, narrow=True)
            ragged(scalar, ACT_RAGGED[:4], ones, dsem)
            scalar.wait_ge(msem, 1)
            bulk(scalar, ACT_BULK, ones, dsem, narrow=False)
            ragged(scalar, ACT_RAGGED[4:], ones, dsem)

        @block.gpsimd
        def _(gpsimd: bass.BassGpSimd):
            # SWDGE probe: one plain pure-ones rectangle (block 30).
            gpsimd.wait_ge(m1, 1)
            bulk(gpsimd, GP_BULK, ones, dsem)

    nc.finalize()
    return nc


def _make_runner():
    """Compile-once runner: jit(shard_map(_body)) over 8 cores with the
    output aliased to the (donated) input — mirrors
    bass2jax.run_bass_via_pjrt, plus lowering_input_output_aliases."""
    global _cached
    if _cached is not None:
        return _cached

    import jax
    from jax.sharding import Mesh, PartitionSpec
    from jax.experimental.shard_map import shard_map
    from concourse import bass2jax

    bass2jax.install_neuronx_cc_hook()
    nc = _build()

    def _body(xg):
        outs = bass2jax._bass_exec_p.bind(
            xg,
            out_avals=(jax.core.ShapedArray((S, S), np.float32),),
            in_names=("x",),
            out_names=("out",),
            lowering_input_output_aliases=((0, 0),),
            sim_require_finite=True,
            sim_require_nnan=True,
            nc=nc,
        )
        return tuple(outs)

    devices = jax.devices()[:N_CORES]
    assert len(devices) == N_CORES, f"need {N_CORES} devices, have {len(devices)}"
    mesh = Mesh(np.asarray(devices), ("core",))
    sharded = jax.jit(
        shard_map(
            _body,
            mesh=mesh,
            in_specs=(PartitionSpec("core"),),
            out_specs=(PartitionSpec("core"),),
            check_rep=False,
        ),
        donate_argnums=(0,),
        keep_unused=True,
    )
    _cached = (nc, sharded)
    return _cached


class _Result:
    def __init__(self, exec_time_ns=None, mean_exec_time_ns=None):
        self.exec_time_ns = exec_time_ns
        self.mean_exec_time_ns = mean_exec_time_ns


def _run(x_full: np.ndarray, trace: bool = False):
    nc, sharded = _make_runner()
    x_full = np.asarray(x_full, dtype=np.float32)
    xg = np.ascontiguousarray(x_full.reshape(N_CORES * S, S))

    if not trace:
        out = sharded(xg)[0]
        return np.asarray(out).reshape(N_CORES, S, S), _Result()

    # Trace path (test.py only): NTFF profile around the execution, then the
    # same gauge/perfetto pipeline run_bass_kernel_spmd uses under axon.
    import glob
    import os
    import tempfile

    from antenv.axon_hooks import get_axon_ntff_profile_hook
    from concourse import bass_utils as BU

    neff_dir = tempfile.mkdtemp()
    hook = get_axon_ntff_profile_hook()
    with hook(neff_dir, [0]):
        out = np.asarray(sharded(xg)[0])

    ntffs = glob.glob(os.path.join(neff_dir, "*_body*.ntff"))
    if not ntffs:
        return out.reshape(N_CORES, S, S), _Result()

    sharepath = BU.upload_artifacts(neff_dir)
    profile = BU.gauge.profiler.Profile(
        profile_path=BU.FishPath(neff_dir),
        kernel_dev_mode=True,
        profile_on_exit=False,
        bass_kernel=nc.m,
        offline_processing=True,
        fname="*_body*",
        annotate_hlo=False,
        metadata={"artifacts_path": sharepath},
    )
    perf = BU._process_ntff_profile(
        profile,
        neff_dir,
        nc,
        list(range(N_CORES)),
        None,
        False,
        {},
        trace_events=False,
    )
    return out.reshape(N_CORES, S, S), _Result(
        perf.exec_time_ns, perf.mean_exec_time_ns
    )


def kernel(x: np.ndarray) -> np.ndarray:
    out, _ = _run(x, trace=False)
    return out
